# revision 1
# baseline (speedup 1.0000x reference)
"""Contrastive loss kernel for Trainium2, 8 NeuronCores, data-parallel over node rows.

Strategy (per core c, shard rows R_c = c*1024 .. c*1024+1024):
  - Normalize the FULL x on-chip (norms via ACT square + DVE 3D-reduce, r = sqrt(1/ss)),
    scale+cast to bf16, DMA-transpose into z1T [256, 8192] (d-major) for the PE.
  - Gram slabs: scores[slab 128 rows, 8192] = z1_shard @ z1^T via PE (bf16, K=256),
    evicted PSUM->SBUF alternating ACT/DVE in [128, 2048] chunks (raw f32).
  - Negative selection: GPSIMD ap_gather with host-prepared index layout.
    ap_gather shares one index list per 16-partition group; the host lays out row p's
    64 indices so they land at output columns [(p%16)*64, (p%16)*64+64). A 16-way
    partition-strided DVE copy extracts each row's own block (the "diagonal" blocks).
  - exp (scale=1/tau) batched on the extracted [128, 8*64], 3D-reduce -> neg_sim.
  - Positives: exact f32 path: dot(x_n, y_n) via DVE mul + 3D-reduce, scaled by
    rx*ry, exp.
  - loss = ln(pos + neg + eps) - ln(pos); per-row losses DMA'd out; host takes the
    global mean over all 8 cores (the unshard step).
"""
import sys

sys.path.insert(0, "/opt/trn_rl_repo")

from contextlib import ExitStack

import numpy as np

import concourse.bacc as bacc
import concourse.mybir as mybir
import concourse.tile as tile
from concourse.bass_utils import run_bass_kernel_spmd

N_NODES = 8192
D = 256
K_NEG = 64
N_CORES = 8
RPC = N_NODES // N_CORES      # rows per core = 1024
SLABS = RPC // 128            # 8 slabs of 128 rows
NT = N_NODES // 128           # 64 x-tiles of [128, 256]
NTS = RPC // 128              # 8 shard tiles
TAU_INV = float(1.0 / (0.5 + 1e-10))
EPS = 1e-5

F32 = mybir.dt.float32
BF16 = mybir.dt.bfloat16
I16 = mybir.dt.int16

_PROG = None  # (nc,) cached compiled program


def _build_program():
    nc = bacc.Bacc("TRN2", target_bir_lowering=False, debug=False,
                   num_devices=N_CORES)

    x_d = nc.dram_tensor("x", [N_NODES, D], F32, kind="ExternalInput")
    xs_d = nc.dram_tensor("xs", [RPC, D], F32, kind="ExternalInput")
    y_d = nc.dram_tensor("y", [RPC, D], F32, kind="ExternalInput")
    idx_d = nc.dram_tensor("idx", [SLABS, 128, K_NEG], I16, kind="ExternalInput")
    loss_d = nc.dram_tensor("loss", [128, SLABS], F32, kind="ExternalOutput")

    AF = mybir.ActivationFunctionType
    ALU = mybir.AluOpType

    with tile.TileContext(nc) as tc, ExitStack() as ctx:
        big = ctx.enter_context(tc.tile_pool(name="big", bufs=1))
        xg_pool = ctx.enter_context(tc.tile_pool(name="xg", bufs=2))
        sqpool = ctx.enter_context(tc.tile_pool(name="sqpool", bufs=2))
        zrow_pool = ctx.enter_context(tc.tile_pool(name="zrow", bufs=6))
        psum = ctx.enter_context(tc.tile_pool(name="psum", bufs=2, space="PSUM"))
        epool = ctx.enter_context(tc.tile_pool(name="epool", bufs=2))

        # ---------------- loads ----------------
        x_r = x_d.ap().rearrange("(t p) d -> p t d", p=128)
        xsbig = big.tile([128, NTS, D], F32)
        nc.sync.dma_start(out=xsbig, in_=xs_d.ap().rearrange("(t p) d -> p t d", p=128))
        ybig = big.tile([128, NTS, D], F32)
        nc.sync.dma_start(out=ybig, in_=y_d.ap().rearrange("(t p) d -> p t d", p=128))
        idx_sb = big.tile([128, SLABS, K_NEG], I16)
        nc.sync.dma_start(out=idx_sb, in_=idx_d.ap().rearrange("s p k -> p s k"))

        G = 8  # tiles per x-group
        # ---------------- shard norms (xs, y) + positive dots ----------------
        SSx = big.tile([128, NTS], F32)
        sqs = sqpool.tile([128, G, D], BF16, tag="sq")
        nc.scalar.activation(sqs[:, 0:NTS, :].rearrange("p a b -> p (a b)"),
                             xsbig.rearrange("p a b -> p (a b)"), AF.Square)
        nc.vector.tensor_reduce(out=SSx, in_=sqs[:, 0:NTS, :],
                                axis=mybir.AxisListType.X, op=ALU.add)
        SSy = big.tile([128, NTS], F32)
        sqy = sqpool.tile([128, G, D], BF16, tag="sq")
        nc.scalar.activation(sqy[:, 0:NTS, :].rearrange("p a b -> p (a b)"),
                             ybig.rearrange("p a b -> p (a b)"), AF.Square)
        nc.vector.tensor_reduce(out=SSy, in_=sqy[:, 0:NTS, :],
                                axis=mybir.AxisListType.X, op=ALU.add)

        SSxi = big.tile([128, NTS], F32)
        nc.vector.reciprocal(SSxi, SSx)
        Rx = big.tile([128, NTS], F32)
        nc.scalar.activation(Rx, SSxi, AF.Sqrt)
        SSyi = big.tile([128, NTS], F32)
        nc.vector.reciprocal(SSyi, SSy)
        Ry = big.tile([128, NTS], F32)
        nc.scalar.activation(Ry, SSyi, AF.Sqrt)

        xyp = xg_pool.tile([128, G, D], F32, tag="xg")
        xy = xyp[:, 0:NTS, :]
        nc.vector.tensor_mul(xy.rearrange("p a b -> p (a b)"),
                             xsbig.rearrange("p a b -> p (a b)"),
                             ybig.rearrange("p a b -> p (a b)"))
        DXY = big.tile([128, NTS], F32)
        nc.vector.tensor_reduce(out=DXY, in_=xy, axis=mybir.AxisListType.X, op=ALU.add)
        # pos_arg = dot * rx * ry ; POS = exp(pos_arg / tau)
        PA = big.tile([128, NTS], F32)
        nc.vector.tensor_mul(PA, DXY, Rx)
        PA2 = big.tile([128, NTS], F32)
        nc.vector.tensor_mul(PA2, PA, Ry)
        POS = big.tile([128, NTS], F32)
        nc.scalar.activation(POS, PA2, AF.Exp, scale=TAU_INV)

        # ---------------- z1sT build ----------------
        z1sT0 = big.tile([128, RPC], BF16)
        z1sT1 = big.tile([128, RPC], BF16)
        zsA = zrow_pool.tile([128, G, 128], BF16, tag="zrA")
        zsB = zrow_pool.tile([128, G, 128], BF16, tag="zrB")
        for t in range(NTS):
            nc.vector.tensor_scalar(out=zsA[:, t, :], in0=xsbig[:, t, 0:128],
                                    scalar1=Rx[:, t:t + 1], scalar2=None,
                                    op0=ALU.mult)
            nc.vector.tensor_scalar(out=zsB[:, t, :], in0=xsbig[:, t, 128:256],
                                    scalar1=Rx[:, t:t + 1], scalar2=None,
                                    op0=ALU.mult)
        nc.sync.dma_start(out=z1sT0.rearrange("p (b q) -> p b q", q=128),
                          in_=zsA.rearrange("p a b -> p (a b)"), transpose=True)
        nc.sync.dma_start(out=z1sT1.rearrange("p (b q) -> p b q", q=128),
                          in_=zsB.rearrange("p a b -> p (a b)"), transpose=True)

        # ---------------- full-x normalize + transpose, streamed in groups ----
        # r[row 128t+p] = 1/sqrt(sum_d x^2) stored at R[p, t]
        z1T0 = big.tile([128, N_NODES], BF16)  # d in [0,128)
        z1T1 = big.tile([128, N_NODES], BF16)  # d in [128,256)
        SS = big.tile([128, NT], F32)
        SSi = big.tile([128, NT], F32)
        R = big.tile([128, NT], F32)
        for g in range(NT // G):
            sl = slice(g * G, (g + 1) * G)
            xg = xg_pool.tile([128, G, D], F32, tag="xg")
            nc.sync.dma_start(out=xg, in_=x_r[:, sl, :])
            sq = sqpool.tile([128, G, D], BF16, tag="sq")
            nc.scalar.activation(sq.rearrange("p a b -> p (a b)"),
                                 xg.rearrange("p a b -> p (a b)"),
                                 AF.Square)
            nc.vector.tensor_reduce(out=SS[:, sl], in_=sq,
                                    axis=mybir.AxisListType.X, op=ALU.add)
            nc.vector.reciprocal(SSi[:, sl], SS[:, sl])
            nc.scalar.activation(R[:, sl], SSi[:, sl], AF.Sqrt)
            zrA = zrow_pool.tile([128, G, 128], BF16, tag="zrA")
            zrB = zrow_pool.tile([128, G, 128], BF16, tag="zrB")
            for tl in range(G):
                t = g * G + tl
                nc.vector.tensor_scalar(out=zrA[:, tl, :], in0=xg[:, tl, 0:128],
                                        scalar1=R[:, t:t + 1], scalar2=None,
                                        op0=ALU.mult)
                nc.vector.tensor_scalar(out=zrB[:, tl, :], in0=xg[:, tl, 128:256],
                                        scalar1=R[:, t:t + 1], scalar2=None,
                                        op0=ALU.mult)
            # batched transpose: one DMA per 128-d half per group
            # (SBUF->SBUF [128, G*128] xbar transpose = G independent 128x128
            #  block transposes)
            nc.sync.dma_start(
                out=z1T0[:, g * G * 128:(g + 1) * G * 128].rearrange(
                    "p (b q) -> p b q", q=128),
                in_=zrA.rearrange("p a b -> p (a b)"), transpose=True)
            nc.sync.dma_start(
                out=z1T1[:, g * G * 128:(g + 1) * G * 128].rearrange(
                    "p (b q) -> p b q", q=128),
                in_=zrB.rearrange("p a b -> p (a b)"), transpose=True)

        # ---------------- Gram slabs + gather ----------------
        Ubig = big.tile([128, SLABS, 1024], F32)  # gather output per slab
        CHUNK = 2048
        for s in range(SLABS):
            E = epool.tile([128, N_NODES], F32, tag="E")
            lhs0 = z1sT0[:, s * 128:(s + 1) * 128]
            lhs1 = z1sT1[:, s * 128:(s + 1) * 128]
            for c4 in range(N_NODES // CHUNK):
                ps = psum.tile([128, CHUNK], F32, tag="ps")
                for j in range(CHUNK // 512):
                    col = c4 * CHUNK + j * 512
                    nc.tensor.matmul(ps[:, j * 512:(j + 1) * 512],
                                     lhsT=lhs0, rhs=z1T0[:, col:col + 512],
                                     start=True, stop=False)
                    nc.tensor.matmul(ps[:, j * 512:(j + 1) * 512],
                                     lhsT=lhs1, rhs=z1T1[:, col:col + 512],
                                     start=False, stop=True)
                dst = E[:, c4 * CHUNK:(c4 + 1) * CHUNK]
                if c4 % 2 == 0:
                    nc.scalar.copy(dst, ps)
                else:
                    nc.vector.tensor_copy(dst, ps)
            nc.gpsimd.ap_gather(out_ap=Ubig[:, s, :], in_ap=E,
                                idxs_ap=idx_sb[:, s, :],
                                channels=128, num_elems=N_NODES, d=1,
                                num_idxs=1024)

        # ---------------- extract own blocks (16 partition-strided DMAs) ----
        # Row p's own gathered values live at Ubig[p, s, (p%16)*64 : +64].
        EX = big.tile([128, SLABS, K_NEG], F32)  # raw selected scores
        for q in range(16):
            nc.sync.dma_start(
                out=EX[q:128:16, :, :],
                in_=Ubig[q:128:16, :, q * K_NEG:(q + 1) * K_NEG])

        # ---------------- exp + reduce + loss ----------------
        EEX = big.tile([128, SLABS, K_NEG], F32)
        nc.scalar.activation(EEX.rearrange("p a b -> p (a b)"),
                             EX.rearrange("p a b -> p (a b)"),
                             AF.Exp, scale=TAU_INV)
        NEG = big.tile([128, SLABS], F32)
        nc.vector.tensor_reduce(out=NEG, in_=EEX, axis=mybir.AxisListType.X,
                                op=ALU.add)
        DEN = big.tile([128, SLABS], F32)
        nc.vector.tensor_add(DEN, NEG, POS)
        DEN2 = big.tile([128, SLABS], F32)
        nc.vector.tensor_scalar_add(DEN2, DEN, EPS)
        LD = big.tile([128, SLABS], F32)
        nc.scalar.activation(LD, DEN2, AF.Ln)
        LP = big.tile([128, SLABS], F32)
        nc.scalar.activation(LP, POS, AF.Ln)
        LOSS = big.tile([128, SLABS], F32)
        nc.vector.tensor_sub(LOSS, LD, LP)
        nc.sync.dma_start(out=loss_d.ap(), in_=LOSS)

    nc.compile()
    return nc


def _get_program():
    global _PROG
    if _PROG is None:
        _PROG = _build_program()
    return _PROG


def _idx_layout(idx_core: np.ndarray) -> np.ndarray:
    """[1024, 64] int -> [SLABS, 128, 64] int16 in ap_gather layout.

    Per slab (128 rows) and 16-partition group g: the gather's shared index
    list must satisfy unwrapped[p*64 + k] = idx[row 16g+p, k], where
    unwrapped[j] = idxs_tile[j % 16, j // 16]. With k = 16t + q this gives
    idxs_tile[q, 4p + t] = idx[16g+p, 16t+q], i.e. the einops rearrange
    'g p (t q) -> (g q) (p t)' per slab.
    """
    out = np.empty((SLABS, 128, K_NEG), dtype=np.int16)
    for s in range(SLABS):
        A = idx_core[s * 128:(s + 1) * 128].astype(np.int16)  # [128, 64]
        A = A.reshape(8, 16, 4, 16)        # [g, p, t, q]
        A = A.transpose(0, 3, 1, 2)        # [g, q, p, t]
        out[s] = A.reshape(128, 64)
    return out


def make_in_maps(x, y, neg_indices):
    in_maps = []
    for c in range(N_CORES):
        lo, hi = c * RPC, (c + 1) * RPC
        in_maps.append({
            "x": np.ascontiguousarray(x, dtype=np.float32),
            "xs": np.ascontiguousarray(x[lo:hi], dtype=np.float32),
            "y": np.ascontiguousarray(y[lo:hi], dtype=np.float32),
            "idx": _idx_layout(neg_indices[lo:hi]),
        })
    return in_maps


def _ensure_ntff_hook():
    """Register the axon NTFF profile hook (missing from this image's antenv).

    Mirrors trn_boot._ntff_profile_via_ctypes over /opt/axon/libaxon_pjrt.so so
    run_bass_kernel_spmd(trace=True) can capture HW profiles.
    """
    import types, ctypes, contextlib, importlib
    try:
        from antenv.axon_hooks import get_axon_ntff_profile_hook  # noqa
        return  # already available
    except ImportError:
        pass
    so_path = "/opt/axon/libaxon_pjrt.so"
    import os
    if not os.path.exists(so_path):
        return
    lib = ctypes.CDLL(so_path)
    if not hasattr(lib, "axon_start_nrt_profile"):
        return
    lib.axon_start_nrt_profile.argtypes = [ctypes.POINTER(ctypes.c_int64),
                                           ctypes.c_size_t]
    lib.axon_start_nrt_profile.restype = ctypes.c_int64
    lib.axon_stop_nrt_profile.argtypes = [ctypes.c_char_p]
    lib.axon_stop_nrt_profile.restype = ctypes.c_int64

    @contextlib.contextmanager
    def _hook(output_dir, device_ids):
        import jax
        jax.devices()
        if device_ids:
            ids = (ctypes.c_int64 * len(device_ids))(*device_ids)
            rc = lib.axon_start_nrt_profile(ids, len(device_ids))
        else:
            rc = lib.axon_start_nrt_profile(None, 0)
        if rc != 0:
            raise RuntimeError(f"axon_start_nrt_profile rc={rc}")
        try:
            yield
        finally:
            n = lib.axon_stop_nrt_profile(str(output_dir).encode())
            if n < 0:
                raise RuntimeError(f"axon_stop_nrt_profile rc={n}")
            print(f"profile: {n} file(s) written to {output_dir}")

    mod = types.ModuleType("antenv.axon_hooks")
    _state = {"hook": _hook}
    mod.get_axon_ntff_profile_hook = lambda: _state["hook"]
    mod.set_axon_ntff_profile_hook = lambda h: _state.update(hook=h)
    import antenv
    sys.modules["antenv.axon_hooks"] = mod
    antenv.axon_hooks = mod


def run_spmd(in_maps, trace=False, **kw):
    nc = _get_program()
    if trace:
        _ensure_ntff_hook()
    return run_bass_kernel_spmd(nc, in_maps, list(range(N_CORES)), trace=trace, **kw)


def kernel(x, y, neg_indices):
    x = np.asarray(x)
    y = np.asarray(y)
    neg_indices = np.asarray(neg_indices)
    res = run_spmd(make_in_maps(x, y, neg_indices)).results
    losses = np.stack([res[c]["loss"] for c in range(N_CORES)])  # [8, 128, SLABS]
    return np.float32(losses.mean())



# revision 5
# speedup vs baseline: 2.5789x; 2.5789x over previous
"""Contrastive loss kernel for Trainium2, 8 NeuronCores, data-parallel over node rows.

Strategy (per core c, shard rows R_c = c*1024 .. c*1024+1024), gather-free:
  - Host pre-casts x to bf16 and uploads a per-core log-mask
    lnm[p, s, c] = 0 if c is a negative of row (c*1024 + s*128 + p) else -20,
    stored as fp8_e4m3 (exact for both values).  Since top-k indices are
    distinct within a row, the mask is binary.
  - On-chip: normalize full x (ACT square + DVE reduce + recip/sqrt), scale to
    z in bf16 (split ACT/DVE), DMA-transpose into z1T [256, 8192] for the PE.
  - Slab loop (8 slabs x 128 rows): Gram chunk [128, 2048] = z1s @ z1^T on PE
    (bf16, K=256).  Then the selection is FUSED arithmetic, no gather:
        DVE: Y = G_psum + lnm_chunk          (PSUM eviction + mask in one pass)
        ACT: ex = exp(2*Y), accum_out += row-sum   (exp + reduce in one pass)
    Unselected columns contribute exp(2*(sim-20)) ~ e^-38 ~ 0.
  - Positives: per-tile fused dot via DVE tensor_tensor_reduce on bf16
    shard tiles; pos = exp(2 * dot * rx * ry); ln(pos) taken analytically.
  - loss = ln(pos + neg + eps) - 2*pos_arg; host averages the 8 cores' rows.
"""
import sys

sys.path.insert(0, "/opt/trn_rl_repo")

from contextlib import ExitStack

import numpy as np
import ml_dtypes

import concourse.bacc as bacc
import concourse.mybir as mybir
import concourse.tile as tile
from concourse.bass_utils import run_bass_kernel_spmd

N_NODES = 8192
D = 256
K_NEG = 64
N_CORES = 8
RPC = N_NODES // N_CORES      # rows per core = 1024
SLABS = RPC // 128            # 8 slabs of 128 rows
NT = N_NODES // 128           # 64 x-tiles of [128, 256]
NTS = RPC // 128              # 8 shard tiles
G = 8                         # tiles per x-group
CHUNK = 2048                  # Gram eviction chunk (4 PSUM banks)
NCH = N_NODES // CHUNK        # 4 chunks per slab
TAU_INV = float(1.0 / (0.5 + 1e-10))
EPS = 1e-5
MASK_OFF = -20.0              # ln-mask "minus infinity"

F32 = mybir.dt.float32
BF16 = mybir.dt.bfloat16
MASK_DT = mybir.dt.float8e4
MASK_NP = ml_dtypes.float8_e4m3

_PROG = None


def _build_program():
    nc = bacc.Bacc("TRN2", target_bir_lowering=False, debug=False,
                   num_devices=N_CORES)

    xb_d = nc.dram_tensor("xb", [N_NODES, D], BF16, kind="ExternalInput")
    xs_d = nc.dram_tensor("xs", [RPC, D], BF16, kind="ExternalInput")
    ys_d = nc.dram_tensor("ys", [RPC, D], BF16, kind="ExternalInput")
    lnm_d = nc.dram_tensor("lnm", [128, SLABS * N_NODES], MASK_DT,
                           kind="ExternalInput")
    loss_d = nc.dram_tensor("loss", [128, SLABS], F32, kind="ExternalOutput")

    AF = mybir.ActivationFunctionType
    ALU = mybir.AluOpType

    with tile.TileContext(nc) as tc, ExitStack() as ctx:
        big = ctx.enter_context(tc.tile_pool(name="big", bufs=1))
        xg_pool = ctx.enter_context(tc.tile_pool(name="xg", bufs=3))
        sqpool = ctx.enter_context(tc.tile_pool(name="sqpool", bufs=2))
        zrow_pool = ctx.enter_context(tc.tile_pool(name="zrow", bufs=3))
        mpool = ctx.enter_context(tc.tile_pool(name="mpool", bufs=3))
        ypool = ctx.enter_context(tc.tile_pool(name="ypool", bufs=3))
        expool = ctx.enter_context(tc.tile_pool(name="expool", bufs=2))
        psum = ctx.enter_context(tc.tile_pool(name="psum", bufs=2, space="PSUM"))

        # ---------------- shard loads ----------------
        xsb = big.tile([128, NTS, D], BF16)
        nc.sync.dma_start(out=xsb, in_=xs_d.ap().rearrange("(t p) d -> p t d", p=128))
        ysb = big.tile([128, NTS, D], BF16)
        nc.sync.dma_start(out=ysb, in_=ys_d.ap().rearrange("(t p) d -> p t d", p=128))

        # ---------------- shard norms + positive dots ----------------
        SSx = big.tile([128, NTS], F32)
        sqs = sqpool.tile([128, G, D], BF16, tag="sq")
        nc.scalar.activation(sqs[:, 0:NTS, :].rearrange("p a b -> p (a b)"),
                             xsb.rearrange("p a b -> p (a b)"), AF.Square)
        nc.vector.tensor_reduce(out=SSx, in_=sqs[:, 0:NTS, :],
                                axis=mybir.AxisListType.X, op=ALU.add)
        SSy = big.tile([128, NTS], F32)
        sqy = sqpool.tile([128, G, D], BF16, tag="sq")
        nc.scalar.activation(sqy[:, 0:NTS, :].rearrange("p a b -> p (a b)"),
                             ysb.rearrange("p a b -> p (a b)"), AF.Square)
        nc.vector.tensor_reduce(out=SSy, in_=sqy[:, 0:NTS, :],
                                axis=mybir.AxisListType.X, op=ALU.add)

        SSxi = big.tile([128, NTS], F32)
        nc.vector.reciprocal(SSxi, SSx)
        Rx = big.tile([128, NTS], F32)
        nc.scalar.activation(Rx, SSxi, AF.Sqrt)
        SSyi = big.tile([128, NTS], F32)
        nc.vector.reciprocal(SSyi, SSy)
        Ry = big.tile([128, NTS], F32)
        nc.scalar.activation(Ry, SSyi, AF.Sqrt)

        # positive dots: bf16 elementwise product + f32 reduce
        xyp = sqpool.tile([128, G, D], BF16, tag="sq")
        nc.vector.tensor_mul(xyp[:, 0:NTS, :].rearrange("p a b -> p (a b)"),
                             xsb.rearrange("p a b -> p (a b)"),
                             ysb.rearrange("p a b -> p (a b)"))
        DXY = big.tile([128, NTS], F32)
        nc.vector.tensor_reduce(out=DXY, in_=xyp[:, 0:NTS, :],
                                axis=mybir.AxisListType.X, op=ALU.add)
        PA = big.tile([128, NTS], F32)
        nc.vector.tensor_mul(PA, DXY, Rx)
        PA2 = big.tile([128, NTS], F32)
        nc.vector.tensor_mul(PA2, PA, Ry)
        POS = big.tile([128, NTS], F32)
        nc.scalar.activation(POS, PA2, AF.Exp, scale=TAU_INV)

        # ---------------- z1sT build (shard lhsT) ----------------
        z1sT0 = big.tile([128, RPC], BF16)
        z1sT1 = big.tile([128, RPC], BF16)
        zsA = zrow_pool.tile([128, G, 128], BF16, tag="zrA")
        zsB = zrow_pool.tile([128, G, 128], BF16, tag="zrB")
        for t in range(NTS):
            nc.scalar.activation(zsA[:, t, :], xsb[:, t, 0:128], AF.Copy,
                                 scale=Rx[:, t:t + 1])
            nc.vector.tensor_scalar(out=zsB[:, t, :], in0=xsb[:, t, 128:256],
                                    scalar1=Rx[:, t:t + 1], scalar2=None,
                                    op0=ALU.mult)
        nc.sync.dma_start(out=z1sT0.rearrange("p (b q) -> p b q", q=128),
                          in_=zsA.rearrange("p a b -> p (a b)"), transpose=True)
        nc.sync.dma_start(out=z1sT1.rearrange("p (b q) -> p b q", q=128),
                          in_=zsB.rearrange("p a b -> p (a b)"), transpose=True)

        # ---------------- full-x normalize + transpose, streamed in groups ----
        x_r = xb_d.ap().rearrange("(t p) d -> p t d", p=128)
        z1T0 = big.tile([128, N_NODES], BF16)  # d in [0,128)
        z1T1 = big.tile([128, N_NODES], BF16)  # d in [128,256)
        SS = big.tile([128, NT], F32)
        SSi = big.tile([128, NT], F32)
        R = big.tile([128, NT], F32)
        for g in range(NT // G):
            sl = slice(g * G, (g + 1) * G)
            xg = xg_pool.tile([128, G, D], BF16, tag="xg")
            nc.sync.dma_start(out=xg, in_=x_r[:, sl, :])
            sq = sqpool.tile([128, G, D], BF16, tag="sq")
            nc.scalar.activation(sq.rearrange("p a b -> p (a b)"),
                                 xg.rearrange("p a b -> p (a b)"),
                                 AF.Square)
            nc.vector.tensor_reduce(out=SS[:, sl], in_=sq,
                                    axis=mybir.AxisListType.X, op=ALU.add)
            nc.vector.reciprocal(SSi[:, sl], SS[:, sl])
            nc.scalar.activation(R[:, sl], SSi[:, sl], AF.Sqrt)
            zrA = zrow_pool.tile([128, G, 128], BF16, tag="zrA")
            zrB = zrow_pool.tile([128, G, 128], BF16, tag="zrB")
            for tl in range(G):
                t = g * G + tl
                # split the scaling across ACT (first half) and DVE (second)
                nc.scalar.activation(zrA[:, tl, :], xg[:, tl, 0:128], AF.Copy,
                                     scale=R[:, t:t + 1])
                nc.vector.tensor_scalar(out=zrB[:, tl, :], in0=xg[:, tl, 128:256],
                                        scalar1=R[:, t:t + 1], scalar2=None,
                                        op0=ALU.mult)
            nc.sync.dma_start(
                out=z1T0[:, g * G * 128:(g + 1) * G * 128].rearrange(
                    "p (b q) -> p b q", q=128),
                in_=zrA.rearrange("p a b -> p (a b)"), transpose=True)
            nc.sync.dma_start(
                out=z1T1[:, g * G * 128:(g + 1) * G * 128].rearrange(
                    "p (b q) -> p b q", q=128),
                in_=zrB.rearrange("p a b -> p (a b)"), transpose=True)

        # ---------------- slab loop: Gram + fused mask/exp/reduce ----------
        NEGC = big.tile([128, SLABS * NCH], F32)  # per-chunk partial sums
        for s in range(SLABS):
            msk = mpool.tile([128, N_NODES], MASK_DT, tag="m")
            nc.sync.dma_start(out=msk,
                              in_=lnm_d.ap()[:, s * N_NODES:(s + 1) * N_NODES])
            lhs0 = z1sT0[:, s * 128:(s + 1) * 128]
            lhs1 = z1sT1[:, s * 128:(s + 1) * 128]
            for c4 in range(NCH):
                ps = psum.tile([128, CHUNK], F32, tag="ps")
                for j in range(CHUNK // 512):
                    col = c4 * CHUNK + j * 512
                    nc.tensor.matmul(ps[:, j * 512:(j + 1) * 512],
                                     lhsT=lhs0, rhs=z1T0[:, col:col + 512],
                                     start=True, stop=False)
                    nc.tensor.matmul(ps[:, j * 512:(j + 1) * 512],
                                     lhsT=lhs1, rhs=z1T1[:, col:col + 512],
                                     start=False, stop=True)
                Y = ypool.tile([128, CHUNK], BF16, tag="y")
                nc.vector.tensor_tensor(
                    out=Y, in0=ps, in1=msk[:, c4 * CHUNK:(c4 + 1) * CHUNK],
                    op=ALU.add)
                ex = expool.tile([128, CHUNK], BF16, tag="ex")
                nc.scalar.activation(ex, Y, AF.Exp, scale=TAU_INV,
                                     accum_out=NEGC[:, s * NCH + c4:s * NCH + c4 + 1])

        # ---------------- loss assembly ----------------
        NEG = big.tile([128, SLABS], F32)
        nc.vector.tensor_reduce(out=NEG, in_=NEGC.rearrange("p (s c) -> p s c", c=NCH),
                                axis=mybir.AxisListType.X,
                                op=ALU.add)
        DEN = big.tile([128, SLABS], F32)
        nc.vector.tensor_add(DEN, NEG, POS)
        DEN2 = big.tile([128, SLABS], F32)
        nc.vector.tensor_scalar_add(DEN2, DEN, EPS)
        LD = big.tile([128, SLABS], F32)
        nc.scalar.activation(LD, DEN2, AF.Ln)
        LP = big.tile([128, SLABS], F32)
        nc.vector.tensor_scalar_mul(LP, PA2, TAU_INV)
        LOSS = big.tile([128, SLABS], F32)
        nc.vector.tensor_sub(LOSS, LD, LP)
        nc.sync.dma_start(out=loss_d.ap(), in_=LOSS)

    nc.compile()
    return nc


def _get_program():
    global _PROG
    if _PROG is None:
        _PROG = _build_program()
    return _PROG


def _make_mask(idx_core: np.ndarray) -> np.ndarray:
    """[1024, 64] int -> [128, SLABS*8192] fp8 ln-mask (p-major layout)."""
    idxc = idx_core.reshape(SLABS, 128, K_NEG).transpose(1, 0, 2)  # [p, s, k]
    lnm = np.full((128, SLABS, N_NODES), MASK_OFF, dtype=np.float32)
    pp = np.arange(128)[:, None, None]
    ss = np.arange(SLABS)[None, :, None]
    lnm[pp, ss, idxc] = 0.0
    return lnm.reshape(128, SLABS * N_NODES).astype(MASK_NP)


def make_in_maps(x, y, neg_indices):
    xb = np.ascontiguousarray(x).astype(ml_dtypes.bfloat16)
    in_maps = []
    for c in range(N_CORES):
        lo, hi = c * RPC, (c + 1) * RPC
        in_maps.append({
            "xb": xb,
            "xs": xb[lo:hi],
            "ys": np.ascontiguousarray(y[lo:hi]).astype(ml_dtypes.bfloat16),
            "lnm": _make_mask(neg_indices[lo:hi]),
        })
    return in_maps


def _ensure_ntff_hook():
    """Register the axon NTFF profile hook (missing from this image's antenv)."""
    import types, ctypes, contextlib
    try:
        from antenv.axon_hooks import get_axon_ntff_profile_hook  # noqa
        return
    except ImportError:
        pass
    so_path = "/opt/axon/libaxon_pjrt.so"
    import os
    if not os.path.exists(so_path):
        return
    lib = ctypes.CDLL(so_path)
    if not hasattr(lib, "axon_start_nrt_profile"):
        return
    lib.axon_start_nrt_profile.argtypes = [ctypes.POINTER(ctypes.c_int64),
                                           ctypes.c_size_t]
    lib.axon_start_nrt_profile.restype = ctypes.c_int64
    lib.axon_stop_nrt_profile.argtypes = [ctypes.c_char_p]
    lib.axon_stop_nrt_profile.restype = ctypes.c_int64

    @contextlib.contextmanager
    def _hook(output_dir, device_ids):
        import jax
        jax.devices()
        if device_ids:
            ids = (ctypes.c_int64 * len(device_ids))(*device_ids)
            rc = lib.axon_start_nrt_profile(ids, len(device_ids))
        else:
            rc = lib.axon_start_nrt_profile(None, 0)
        if rc != 0:
            raise RuntimeError(f"axon_start_nrt_profile rc={rc}")
        try:
            yield
        finally:
            n = lib.axon_stop_nrt_profile(str(output_dir).encode())
            if n < 0:
                raise RuntimeError(f"axon_stop_nrt_profile rc={n}")
            print(f"profile: {n} file(s) written to {output_dir}")

    mod = types.ModuleType("antenv.axon_hooks")
    _state = {"hook": _hook}
    mod.get_axon_ntff_profile_hook = lambda: _state["hook"]
    mod.set_axon_ntff_profile_hook = lambda h: _state.update(hook=h)
    import antenv
    sys.modules["antenv.axon_hooks"] = mod
    antenv.axon_hooks = mod


def run_spmd(in_maps, trace=False, **kw):
    nc = _get_program()
    if trace:
        _ensure_ntff_hook()
    return run_bass_kernel_spmd(nc, in_maps, list(range(N_CORES)), trace=trace, **kw)


def kernel(x, y, neg_indices):
    x = np.asarray(x)
    y = np.asarray(y)
    neg_indices = np.asarray(neg_indices)
    res = run_spmd(make_in_maps(x, y, neg_indices)).results
    losses = np.stack([res[c]["loss"] for c in range(N_CORES)])  # [8, 128, SLABS]
    return np.float32(losses.mean())
